# revision 19
# baseline (speedup 1.0000x reference)
"""Trainium2 Bass kernel for BaseAttentionConvolution (7x7 neighborhood attention).

Computation (reference, fp32):
    q = Q @ Wq + bq                     # [B,H,W,64]
    k = K @ Wk + bk                     # [B,H,W,64]
    S[p, (dy,dx)] = q[p] . k[p+(dy,dx)]         (7x7 window, -inf outside image)
    P = softmax(S / 8)
    O[p] = sum_j P[p,j] * V[p+j]        # [B,H,W,128]
    out = relu(O @ Wv + bv)             # [B,H,W,128]

Sharding: B*H = 192 rows split into 8 bands of 24 rows (one per core).

Device computes the unnormalized attention output projected through Wv
(opT = (sum_j E_j V_j) @ Wv, transposed) plus the softmax denominators;
the cheap final normalization out = relu(opT * (1/den) + bv) runs on the
host (exact: relu(z*r) = r*relu(z) for r > 0, and den > 0 always).

Per-core inputs (SPMD program, per-core data, bf16 matmul path):
  - qt  [128, 2304]  Q rows transposed to channel-major
  - kt  [128, 2912]  K rows + 3-row halo, zero-padded (+32 pad cols)
  - v   [96, 30, 128] V rows + halo, pixel-in-row major
  - cb  [128, 641]   bf16 const blob: wq | wk | wv | b4 mask | ones
  - cf  [64, 2]      fp32 const blob: bq | bk

On-chip (keys-on-partitions, fp32 PSUM accumulation):
  qT[64, 2304] = Wq^T @ qt (+bq), kT[64, 2912] = Wk^T @ kt (+bk)   on PE
  bands of 4 query rows in interleaved PAIRS (hides the PE->ACT->DVE->PE
  chain); per (band, k-row r = band*4+i), trimmed to the valid 96-col
  query blocks (|qrow-krow|<=3):
    S[96k, sub] = kT_r^T . qT_band_sub          (PE)
    E[96, sub]  = exp(S/8)                      (ACT, bf16)
    E *= b4                                     (DVE, banded dx mask)
    den[1, 384] += ones^T . E                   (PE, col group 3)
    outT[128e, 384] += V_r^T . E                (PE)
  finalize per band: den -> SBUF slab; oT = bf16(outT); ONE matmul
  opT[128, 384] = Wv^T @ oT into PSUM; DMA opT rows straight to DRAM.
  Out-of-image k-rows contribute exp(0)*b4 to den (V rows are zero), which
  the host subtracts (precomputed `excess`) before taking reciprocals.
"""

import numpy as np
from contextlib import ExitStack

import ml_dtypes

import concourse.bass as bass
import concourse.bacc as bacc
import concourse.tile as tile
from concourse import mybir
from concourse.bass_utils import run_bass_kernel_spmd

DT = mybir.dt.float32
BF = mybir.dt.bfloat16
AF = mybir.ActivationFunctionType
ALU = mybir.AluOpType

# Problem constants (hardcoded per contract)
B, H, W, C, KD, OD = 2, 96, 96, 128, 64, 128
KS, PAD = 7, 3
NCORES = 8
ROWS = (B * H) // NCORES        # 24 query rows per core
KROWS = ROWS + 2 * PAD          # 30 k/v rows per core (with halo)
NQ = ROWS * W                   # 2304 query pixels per core
NK = KROWS * W                  # 2880 key pixels per core
NKP = NK + 32                   # kt padded for uniform 512-col projection tiles
BAND = 4                        # query rows per band
NBANDS = ROWS // BAND           # 6
BN = BAND * W                   # 384 band query columns
NKR = BAND + 2 * PAD            # 10 k-rows per band
SCALE = 1.0 / np.sqrt(KD)       # 1/8
CBW = KD + KD + OD + BN + 1     # bf16 const blob width (641)

MM_DTYPE = "bf16"
KEEPWARM = True


def build_nc(mm_dtype=MM_DTYPE, with_bv=False, reps=1):
    nc = bacc.Bacc(None, target_bir_lowering=False)
    qt = nc.dram_tensor("qt", [C, NQ], BF, kind="ExternalInput")
    kt = nc.dram_tensor("kt", [C, NKP], BF, kind="ExternalInput")
    v = nc.dram_tensor("v", [W, KROWS, C], BF, kind="ExternalInput")
    cb = nc.dram_tensor("cb", [C, CBW], BF, kind="ExternalInput")
    cf = nc.dram_tensor("cf", [KD, 2], DT, kind="ExternalInput")
    out = nc.dram_tensor("out", [ROWS, OD, W], DT, kind="ExternalOutput")
    dout = nc.dram_tensor("dout", [1, NQ], DT, kind="ExternalOutput")

    with tile.TileContext(nc) as tc, ExitStack() as ctx:
        consts = ctx.enter_context(tc.tile_pool(name="consts", bufs=1))
        slabs = ctx.enter_context(tc.tile_pool(name="slabs", bufs=1))
        e_pool = ctx.enter_context(tc.tile_pool(name="e_pool", bufs=8))
        o_pool = ctx.enter_context(tc.tile_pool(name="o_pool", bufs=2))
        outs = ctx.enter_context(tc.tile_pool(name="outs", bufs=2))
        ps_a = ctx.enter_context(tc.tile_pool(name="ps_a", bufs=4, space="PSUM"))
        ps_o = ctx.enter_context(tc.tile_pool(name="ps_o", bufs=2, space="PSUM"))
        ps_d = ctx.enter_context(tc.tile_pool(name="ps_d", bufs=2, space="PSUM"))

        for _rep in range(reps):
            # ---- input DMAs (const blob first: projections need wq/wk) ----
            cb_s = consts.tile([C, CBW], BF, tag="ccb")
            nc.sync.dma_start(out=cb_s[:], in_=cb[:])
            qt_s = slabs.tile([C, NQ], BF, tag="sqt")
            nc.sync.dma_start(out=qt_s[:], in_=qt[:])
            cf_s = consts.tile([KD, 2], DT, tag="ccf")
            nc.sync.dma_start(out=cf_s[:], in_=cf[:])
            kt_s = slabs.tile([C, NKP], BF, tag="skt")
            nc.sync.dma_start(out=kt_s[:], in_=kt[:])
            v_s = slabs.tile([W, KROWS, C], BF, tag="sv")
            nc.sync.dma_start(out=v_s[:], in_=v[:])

            wq_s = cb_s[:, 0:KD]
            wk_s = cb_s[:, KD : 2 * KD]
            wv_s = cb_s[:, 2 * KD : 2 * KD + OD]
            b4_s = cb_s[0:W, 2 * KD + OD : 2 * KD + OD + BN]
            ones96 = cb_s[0:W, CBW - 1 : CBW]
            bq_s = cf_s[:, 0:1]
            bk_s = cf_s[:, 1:2]

            den_all = slabs.tile([1, NQ], DT, tag="sden")

            # ---- PE warm-up: dependency-free matmuls while DMAs land, so
            # the HAM clock gate reaches 8/8 before the real stream ----
            warm = slabs.tile([C, 512], BF, tag="swarm")
            nc.vector.memset(warm[:], 0.0)
            for _w in range(12):
                wps = ps_a.tile([C, 512], DT, tag="w")
                nc.tensor.matmul(
                    out=wps[:], lhsT=warm[:, :C], rhs=warm[:],
                    start=True, stop=True,
                )

            # ---- projections: all q tiles, warm burst over the kt DMA
            # wait, then all k tiles (keeps the PE busy + HAM warm) ----
            def emit_warm(n, cols=512):
                for _w2 in range(n):
                    wps2 = ps_a.tile([C, 512], DT, tag="w", name="wps2")
                    nc.tensor.matmul(
                        out=wps2[:, :cols], lhsT=warm[:, :C],
                        rhs=warm[:, :cols], start=True, stop=True,
                    )

            qT_s = slabs.tile([KD, NQ], BF, tag="sqT")
            kT_s = slabs.tile([KD, NKP], BF, tag="skT")
            jobs = []
            for j0 in range(0, NQ, 512):
                jobs.append((qT_s, qt_s, wq_s, bq_s, min(j0 + 512, NQ), j0, False))
            jobs.append(None)  # warm burst while kt lands
            for j0 in range(0, NKP, 512):
                jobs.append((kT_s, kt_s, wk_s, bk_s, min(j0 + 512, NKP), j0, True))
            for job in jobs:
                if job is None:
                    emit_warm(13)
                    continue
                dst, src_t, wmat, bvec, j1, j0, on_act = job
                ps = ps_a.tile([C, 512], DT, tag="w")
                nc.tensor.matmul(
                    out=ps[:KD, : j1 - j0],
                    lhsT=wmat,
                    rhs=src_t[:, j0:j1],
                    start=True,
                    stop=True,
                )
                if on_act:
                    nc.scalar.activation(
                        dst[:, j0:j1], ps[:KD, : j1 - j0], AF.Identity,
                        bias=bvec, scale=1.0,
                    )
                else:
                    nc.vector.tensor_scalar_add(
                        dst[:, j0:j1], ps[:KD, : j1 - j0], bvec
                    )

            emit_warm(4)

            # ---- bands, processed as interleaved pairs ----
            state = {}
            pending_finalize = []

            def front(band, i):
                """S matmul + exp + mask for k-row i of `band`."""
                st = state[band]
                h0 = band * BAND
                r = h0 + i
                c_lo, c_hi = max(0, i - 6), min(3, i)
                lo, hi = c_lo * W, (c_hi + 1) * W
                S = ps_a.tile([C, 512], DT, tag="w")
                nc.tensor.matmul(
                    out=S[:W, lo:hi],
                    lhsT=kT_s[:, r * W : (r + 1) * W],
                    rhs=qT_s[:, h0 * W + lo : h0 * W + hi],
                    start=True,
                    stop=True,
                )
                if KEEPWARM:
                    nc.tensor.matmul(
                        out=S[:, BN : BN + 128],
                        lhsT=warm[:, :C],
                        rhs=warm[:, :128],
                        start=True,
                        stop=True,
                    )
                E = e_pool.tile([W, BN], BF, tag="E")
                if i == 0:
                    nc.gpsimd.memset(E[:, hi:BN], 0.0)
                elif i == NKR - 1:
                    nc.gpsimd.memset(E[:, 0:lo], 0.0)
                nc.scalar.activation(
                    E[:, lo:hi], S[:W, lo:hi], AF.Exp, bias=0.0, scale=SCALE
                )
                nc.vector.tensor_mul(E[:, lo:hi], E[:, lo:hi], b4_s[:, lo:hi])
                st["E"][i] = E

            def back(band, i):
                """den/outT accumulation for k-row i of `band`."""
                st = state[band]
                h0 = band * BAND
                r = h0 + i
                c_lo, c_hi = max(0, i - 6), min(3, i)
                full = i == 0 or i == NKR - 1
                lo, hi = (0, BN) if full else (c_lo * W, (c_hi + 1) * W)
                E = st["E"][i]
                # den in PE column group 3 (output partition 96) so it can
                # overlap the adjacent S matmul (col groups 0-2).
                nc.tensor.matmul(
                    out=st["den"][W : W + 1, lo:hi],
                    lhsT=ones96,
                    rhs=E[:, lo:hi],
                    start=(i == 0),
                    stop=(i == NKR - 1),
                    tile_position=(0, W),
                )
                nc.tensor.matmul(
                    out=st["outT"][:, lo:hi],
                    lhsT=v_s[:, r, :],
                    rhs=E[:, lo:hi],
                    start=(i == 0),
                    stop=(i == NKR - 1),
                )

            def finalize_copies(band):
                """Drain den/outT PSUM right after the band's last back()."""
                st = state[band]
                h0 = band * BAND
                nc.vector.tensor_copy(
                    den_all[:, h0 * W : h0 * W + BN], st["den"][W : W + 1, 0:BN]
                )
                oT = o_pool.tile([OD, BN], BF, tag="oT")
                nc.vector.tensor_copy(oT[:], st["outT"][:])
                st["oT"] = oT

            def finalize_rest(band):
                """opT = Wv^T @ oT into the band's den psum bank; DMA out."""
                st = state[band]
                h0 = band * BAND
                op = st["den"]
                nc.tensor.matmul(
                    out=op[:, 0:BN],
                    lhsT=wv_s,
                    rhs=st["oT"][:],
                    start=True,
                    stop=True,
                )
                ost = outs.tile([C, BN], DT, tag="ost")
                nc.scalar.activation(ost[:], op[:, 0:BN], AF.Copy)
                for c in range(BAND):
                    nc.sync.dma_start(
                        out=out[h0 + c], in_=ost[:, c * W : (c + 1) * W]
                    )

            DEPTH = 6  # back() runs DEPTH slots behind front()
            for pair in range(NBANDS // 2):
                bands = (2 * pair, 2 * pair + 1)
                for bd in bands:
                    state[bd] = {
                        "E": {},
                        "outT": ps_o.tile([OD, BN], DT, tag="outT", name="outT"),
                        "den": ps_d.tile([C, 512], DT, tag="den", name="den"),
                    }
                slots = [(bd, i) for i in range(NKR) for bd in bands]
                for s, (bd, i) in enumerate(slots):
                    front(bd, i)
                    if s == 1 and pending_finalize:
                        for pbd in pending_finalize:
                            finalize_rest(pbd)
                        pending_finalize.clear()
                    if s >= DEPTH:
                        back(*slots[s - DEPTH])
                for s in range(len(slots) - DEPTH, len(slots)):
                    back(*slots[s])
                for bd in bands:
                    finalize_copies(bd)
                    pending_finalize.append(bd)
            for pbd in pending_finalize:
                finalize_rest(pbd)
            nc.sync.dma_start(out=dout[:], in_=den_all[:])

    nc.compile()
    return nc


def make_in_maps(Q, K, V, Wq, bq, Wk, bk, Wv, bv, mm_dtype=None):
    BFN = ml_dtypes.bfloat16

    Q = np.asarray(Q, np.float32)
    K = np.asarray(K, np.float32)
    V = np.asarray(V, np.float32)

    # bf16 const blob: wq | wk | wv | b4 | ones
    idx = np.arange(W)
    Bm = (np.abs(idx[:, None] - idx[None, :]) <= PAD).astype(np.float32)
    cbf = np.zeros((C, CBW), np.float32)
    cbf[:, 0:KD] = np.asarray(Wq, np.float32)
    cbf[:, KD : 2 * KD] = np.asarray(Wk, np.float32)
    cbf[:, 2 * KD : 2 * KD + OD] = np.asarray(Wv, np.float32)
    cbf[0:W, 2 * KD + OD : 2 * KD + OD + BN] = np.tile(Bm, (1, BAND))
    cbf[0:W, CBW - 1] = 1.0
    cbb = np.ascontiguousarray(cbf).astype(BFN)
    cff = np.zeros((KD, 2), np.float32)
    cff[:, 0] = np.asarray(bq, np.float32).reshape(KD)
    cff[:, 1] = np.asarray(bk, np.float32).reshape(KD)

    in_maps = []
    for core in range(NCORES):
        b = core // (H // ROWS)
        h_start = (core % (H // ROWS)) * ROWS

        qs = Q[b, h_start : h_start + ROWS]
        qtc = np.ascontiguousarray(qs.reshape(NQ, C).T).astype(BFN)

        kpad = np.zeros((KROWS, W, C), np.float32)
        vpad = np.zeros((KROWS, W, C), np.float32)
        for j in range(KROWS):
            g = h_start - PAD + j
            if 0 <= g < H:
                kpad[j] = K[b, g]
                vpad[j] = V[b, g]
        ktc = np.zeros((C, NKP), np.float32)
        ktc[:, :NK] = kpad.reshape(NK, C).T
        ktc = np.ascontiguousarray(ktc).astype(BFN)
        vtc = np.ascontiguousarray(vpad.transpose(1, 0, 2)).astype(BFN)

        in_maps.append(
            {"qt": qtc, "kt": ktc, "v": vtc, "cb": cbb, "cf": cff}
        )
    return in_maps


def gather(results, bv):
    """Host-side epilogue: out = relu(opT/den + bv)."""
    idx = np.arange(W)
    bw = (np.abs(idx[:, None] - idx[None, :]) <= PAD).astype(np.float32).sum(0)
    bvv = np.asarray(bv, np.float32).reshape(1, OD)
    full = np.empty((B, H, W, OD), np.float32)
    for core in range(NCORES):
        b = core // (H // ROWS)
        h_start = (core % (H // ROWS)) * ROWS
        inv = np.array(
            [sum(1 for dy in range(-PAD, PAD + 1)
                 if not (0 <= h_start + c + dy < H))
             for c in range(ROWS)], np.float32)
        den = results[core]["dout"].reshape(ROWS, W) - inv[:, None] * bw[None, :]
        opt = results[core]["out"]  # [ROWS, OD, W]
        o = opt.transpose(0, 2, 1) / den[:, :, None] + bvv
        full[b, h_start : h_start + ROWS] = np.maximum(o, 0.0)
    return full


_NC_CACHE = {}


def get_nc(mm_dtype=MM_DTYPE, with_bv=False, reps=1):
    key = (mm_dtype, with_bv, reps)
    if key not in _NC_CACHE:
        _NC_CACHE[key] = build_nc(mm_dtype=mm_dtype, with_bv=with_bv, reps=reps)
    return _NC_CACHE[key]


def kernel(Q, K, V, Wq, bq, Wk, bk, Wv, bv):
    nc = get_nc(MM_DTYPE, False)
    in_maps = make_in_maps(Q, K, V, Wq, bq, Wk, bk, Wv, bv, mm_dtype=MM_DTYPE)
    res = run_bass_kernel_spmd(nc, in_maps, list(range(NCORES)))
    return gather(res.results, bv)
